# revision 2
# baseline (speedup 1.0000x reference)
"""Trainium2 Bass kernel for nn_Decay (gated decay-memory block).

  gate  = sigmoid(x @ Wg + bg)
  store = (x @ Wv) * gate * scale          scale = sqrt(1 - decay)
  mem   = decay-scan(store)                y_t = store_t + decay * y_{t-1}
  que   = sigmoid(x @ Wq + bq)
  out   = (mem * que * scale) @ Wo

Sharding (8 cores): core c handles batch b = c//2, token half h = c%2
(2048 output tokens each).  The decay scan needs history: each core
computes a 128-token halo before its token range (zero-padded for h=0,
so all cores run the identical program).  decay^128 ~ 1.4e-3, below the
bf16 noise floor.  No collectives.

All matmul operands are bf16 (measured: 512-row bf16 matmul = 216 ns vs
227 ns f32r; 256-row = 109 ns; accumulation stays f32 in PSUM; end-to-end
rel err ~3.5e-3 vs the 2e-2 gate).  x lives RESIDENT in SBUF as bf16
[E, T] (68 KB/partition), so each of the 4 m-quarter phases re-reads it
from SBUF at zero DMA cost — total HBM traffic drops to ~75 MB/core and
the DMA-burst stalls at phase boundaries disappear.

Layout: [feature (partitions), token (free)] everywhere.
 - projections:  out[m_tile, t_blk] = sum_ec Wx[ec, m_tile].T @ xres[ec, t_blk]
 - decay scan: DVE tensor_tensor_scan along the free (token) axis
 - matmul free dim TB=512 (one full PSUM bank) amortizes instruction
   overhead; the 128-token halo block runs as a cheap 128-row matmul.

Phases (x resident, weights double-buffered):
  A0..A3: m-quarter q of {Wv*scale, Wg, Wq} resident; computes
          pv, gate, store, mem(scan), que, l0 -> spill l0 (bf16)
  C:      Wo*scale fully resident in the freed weight buffers;
          outT[e, t] = sum_m Wo.T @ l0, one pass over l0.
"""

import sys

sys.path.insert(0, "/opt/trn_rl_repo")

import numpy as np
import ml_dtypes

import concourse.bass as bass
import concourse.tile as tile
from concourse import bacc, mybir
from concourse.bass_utils import run_bass_kernel_spmd

# Problem constants (hardcoded per harness contract)
B, S, E, M = 4, 4096, 2048, 2048
DECAY = 0.95
SCALE = float(np.sqrt(1.0 - DECAY))

N_CORES = 8
HALO = 128            # halo tokens ahead of each core's range
OUT_T = S // 2        # output tokens per core (2048)
T = OUT_T + HALO      # resident tokens per core (2176)
TB = 512              # token block (matmul free dim, one PSUM bank)
NTB = OUT_T // TB     # 4 output-token blocks
P = 128
EC = E // P           # 16 contraction chunks
MT = M // P           # 16 m tiles
MQ = 4                # m-quarters
MT_Q = MT // MQ       # 4 m-tiles per quarter
MQW = MT_Q * P        # 512
EQ = E // 4           # 512-wide output-e range per Wo tile
F32 = mybir.dt.float32
BF16 = mybir.dt.bfloat16
BF16_NP = ml_dtypes.bfloat16


def build_module(has_bias):
    nc = bacc.Bacc()

    xT_d = nc.dram_tensor("xT", [E, T], BF16, kind="ExternalInput")
    wv_d = nc.dram_tensor("Wvs", [E, M], BF16, kind="ExternalInput")
    wg_d = nc.dram_tensor("Wg", [E, M], BF16, kind="ExternalInput")
    wq_d = nc.dram_tensor("Wq", [E, M], BF16, kind="ExternalInput")
    wo_d = nc.dram_tensor("Wos", [M, E], BF16, kind="ExternalInput")
    if has_bias:
        bg_d = nc.dram_tensor("bg", [M], F32, kind="ExternalInput")
        bq_d = nc.dram_tensor("bq", [M], F32, kind="ExternalInput")
    outT_d = nc.dram_tensor("outT", [E, OUT_T], F32, kind="ExternalOutput")
    l0_buf = nc.dram_tensor("l0_buf", [M, OUT_T], BF16)  # internal spill

    with tile.TileContext(nc) as tc:
        with (
            tc.tile_pool(name="cp", bufs=1) as cp,
            tc.tile_pool(name="wvp", bufs=2) as wvp,
            tc.tile_pool(name="wgp", bufs=2) as wgp,
            tc.tile_pool(name="wqp", bufs=2) as wqp,
            tc.tile_pool(name="wsp", bufs=3) as wsp,
            tc.tile_pool(name="l0p", bufs=4) as l0p,
            tc.tile_pool(name="memp", bufs=2) as memp,
            tc.tile_pool(name="ps", bufs=2, space="PSUM") as ps,
        ):
            # consts: decay broadcast [:, :TB]; bg at [:, TB:TB+MT]; bq after
            consts = cp.tile([P, TB + 2 * MT], F32, tag="consts", name="consts")
            nc.vector.memset(consts[:, 0:TB], DECAY)
            if has_bias:
                nc.sync.dma_start(
                    out=consts[:, TB : TB + MT],
                    in_=bg_d.rearrange("(c p) -> p c", p=P),
                )
                nc.sync.dma_start(
                    out=consts[:, TB + MT : TB + 2 * MT],
                    in_=bq_d.rearrange("(c p) -> p c", p=P),
                )
            decay_t = consts[:, 0:TB]

            xT_r = xT_d.rearrange("(c p) t -> p c t", p=P)
            l0_r = l0_buf.rearrange("(c p) t -> p c t", p=P)
            outT_r = outT_d.rearrange("(c p) t -> p c t", p=P)

            w_tiles = {}

            def load_w(pool, dram, key, q, qname):
                msl = slice(q * MQW, (q + 1) * MQW)
                t = pool.tile([P, EC, MQW], BF16, tag=key, name=f"{qname}{q}")
                nc.scalar.dma_start(
                    out=t, in_=dram[:, msl].rearrange("(c p) m -> p c m", p=P)
                )
                w_tiles[(key, q)] = t

            def load_wo(pool, eq):
                # Wo e-quarter [P, MT, EQ] = 16KB, same size as an A tile
                esl = slice(eq * EQ, (eq + 1) * EQ)
                t = pool.tile([P, MT, EQ], BF16, tag=pool.name[:2], name=f"wo{eq}")
                nc.scalar.dma_start(
                    out=t, in_=wo_d[:, esl].rearrange("(c p) e -> p c e", p=P)
                )
                w_tiles[("o", eq)] = t

            with tc.tile_pool(name="xp", bufs=1) as xp:
                xres = xp.tile([P, EC, T], BF16, tag="xres", name="xres")
                # startup loads, in order of first need:
                nc.sync.dma_start(
                    out=xres[:, :, 0:HALO],
                    in_=xT_r[:, :, 0:HALO],
                )
                load_w(wvp, wv_d, "wv", 0, "wv")
                load_w(wgp, wg_d, "wg", 0, "wg")
                nc.sync.dma_start(
                    out=xres[:, :, HALO : HALO + TB],
                    in_=xT_r[:, :, HALO : HALO + TB],
                )
                load_w(wqp, wq_d, "wq", 0, "wq")
                for b in range(1, NTB):
                    nc.sync.dma_start(
                        out=xres[:, :, HALO + b * TB : HALO + (b + 1) * TB],
                        in_=xT_r[:, :, HALO + b * TB : HALO + (b + 1) * TB],
                    )

                # ---- Phases A0..A3: quarter q of m ----
                for q in range(MQ):
                    wv = w_tiles[("wv", q)]
                    wg = w_tiles[("wg", q)]
                    wq = w_tiles[("wq", q)]

                    # halo block: pv/pg/scan only (cheap 128-row matmuls)
                    mem_h = memp.tile(
                        [P, MT_Q, TB], F32, tag="mem", name=f"memh{q}"
                    )
                    hsl = slice(0, HALO)
                    for mt in range(MT_Q):
                        wsl = slice(mt * P, (mt + 1) * P)
                        pv = ps.tile(
                            [P, TB], F32, tag="pv", bufs=5, name=f"pvh{q}_{mt}"
                        )
                        for ec in range(EC):
                            nc.tensor.matmul(
                                pv[:, hsl], lhsT=wv[:, ec, wsl],
                                rhs=xres[:, ec, hsl],
                                start=(ec == 0), stop=(ec == EC - 1),
                            )
                        pg = ps.tile(
                            [P, TB], F32, tag="pg", bufs=3, name=f"pgh{q}_{mt}"
                        )
                        for ec in range(EC):
                            nc.tensor.matmul(
                                pg[:, hsl], lhsT=wg[:, ec, wsl],
                                rhs=xres[:, ec, hsl],
                                start=(ec == 0), stop=(ec == EC - 1),
                            )
                        mtg = q * MT_Q + mt
                        ws = wsp.tile([P, 3, TB], F32, tag="ws", name=f"wsh{q}_{mt}")
                        gate, store = ws[:, 0, hsl], ws[:, 1, hsl]
                        nc.scalar.activation(
                            gate, pg[:, hsl], mybir.ActivationFunctionType.Sigmoid,
                            bias=consts[:, TB + mtg : TB + mtg + 1] if has_bias else 0.0,
                        )
                        nc.vector.tensor_mul(store, pv[:, hsl], gate)
                        nc.vector.tensor_tensor_scan(
                            mem_h[:, mt, hsl], decay_t[:, hsl], store,
                            initial=0.0,
                            op0=mybir.AluOpType.mult, op1=mybir.AluOpType.add,
                        )
                    mem_prev, prev_last = mem_h, HALO - 1

                    for tb in range(NTB):
                        xsl = slice(HALO + tb * TB, HALO + (tb + 1) * TB)
                        osl = slice(tb * TB, (tb + 1) * TB)
                        # spread next-phase weight loads across the phase
                        if q + 1 < MQ:
                            if tb == 1:
                                load_w(wvp, wv_d, "wv", q + 1, "wv")
                            elif tb == 2:
                                load_w(wgp, wg_d, "wg", q + 1, "wg")
                            elif tb == 3:
                                load_w(wqp, wq_d, "wq", q + 1, "wq")
                        else:
                            # Wo quarters into the A-weight buffers as they free
                            if tb == 1:
                                load_wo(wvp, 0)
                            elif tb == 2:
                                load_wo(wgp, 1)
                            elif tb == 3:
                                load_wo(wqp, 2)
                        mem_t = memp.tile(
                            [P, MT_Q, TB], F32, tag="mem", name=f"mem{q}_{tb}"
                        )
                        pvs = []
                        for mt in range(MT_Q):
                            wsl = slice(mt * P, (mt + 1) * P)
                            pv = ps.tile(
                                [P, TB], F32, tag="pv", bufs=5,
                                name=f"pv{q}_{tb}_{mt}",
                            )
                            for ec in range(EC):
                                nc.tensor.matmul(
                                    pv, lhsT=wv[:, ec, wsl], rhs=xres[:, ec, xsl],
                                    start=(ec == 0), stop=(ec == EC - 1),
                                )
                            pvs.append(pv)
                        wss = []
                        for mt in range(MT_Q):
                            mtg = q * MT_Q + mt
                            wsl = slice(mt * P, (mt + 1) * P)
                            pg = ps.tile(
                                [P, TB], F32, tag="pg", bufs=3,
                                name=f"pg{q}_{tb}_{mt}",
                            )
                            for ec in range(EC):
                                nc.tensor.matmul(
                                    pg, lhsT=wg[:, ec, wsl], rhs=xres[:, ec, xsl],
                                    start=(ec == 0), stop=(ec == EC - 1),
                                )
                            ws = wsp.tile(
                                [P, 3, TB], F32, tag="ws", name=f"ws{q}_{tb}_{mt}"
                            )
                            wss.append(ws)
                            gate, store = ws[:, 0, :], ws[:, 1, :]
                            nc.scalar.activation(
                                gate, pg, mybir.ActivationFunctionType.Sigmoid,
                                bias=consts[:, TB + mtg : TB + mtg + 1]
                                if has_bias else 0.0,
                            )
                            nc.vector.tensor_mul(store, pvs[mt], gate)
                            nc.vector.tensor_tensor_scan(
                                mem_t[:, mt, :], decay_t, store,
                                initial=mem_prev[:, mt, prev_last : prev_last + 1],
                                op0=mybir.AluOpType.mult, op1=mybir.AluOpType.add,
                            )
                        for mt in range(MT_Q):
                            mtg = q * MT_Q + mt
                            wsl = slice(mt * P, (mt + 1) * P)
                            pq = ps.tile(
                                [P, TB], F32, tag="pv", bufs=5,
                                name=f"pq{q}_{tb}_{mt}",
                            )
                            for ec in range(EC):
                                nc.tensor.matmul(
                                    pq, lhsT=wq[:, ec, wsl], rhs=xres[:, ec, xsl],
                                    start=(ec == 0), stop=(ec == EC - 1),
                                )
                            que = wss[mt][:, 2, :]
                            nc.scalar.activation(
                                que, pq, mybir.ActivationFunctionType.Sigmoid,
                                bias=consts[:, TB + MT + mtg : TB + MT + mtg + 1]
                                if has_bias else 0.0,
                            )
                            l0 = l0p.tile([P, TB], BF16, tag="l0",
                                          name=f"l0{q}_{tb}_{mt}")
                            nc.vector.tensor_mul(l0, mem_t[:, mt, :], que)
                            nc.gpsimd.dma_start(
                                out=l0_r[:, mtg : mtg + 1, osl],
                                in_=l0.unsqueeze(1),
                            )
                        mem_prev, prev_last = mem_t, TB - 1

            # ---- Phase C: output projection, Wo fully resident ----
            # wo3 takes the second wv-tag buffer (freed at A3's end); its DMA
            # overlaps the first three e-quarters' matmuls at tb=0.
            load_wo(wvp, 3)
            with tc.tile_pool(name="ltp", bufs=3) as ltp:
                for tb in range(NTB):
                    osl = slice(tb * TB, (tb + 1) * TB)
                    lt = ltp.tile([P, MT, TB], BF16, tag="lt", name=f"lt{tb}")
                    nc.sync.dma_start(out=lt, in_=l0_r[:, :, osl])
                    for j in range(4):
                        wo = w_tiles[("o", j)]
                        ot = memp.tile([P, 4, TB], F32, tag="mem", name=f"ot{tb}_{j}")
                        for et in range(4):
                            po = ps.tile(
                                [P, TB], F32, tag="pg", bufs=3,
                                name=f"po{tb}_{j}_{et}",
                            )
                            for mc in range(MT):
                                nc.tensor.matmul(
                                    po, lhsT=wo[:, mc, et * P : (et + 1) * P],
                                    rhs=lt[:, mc, :],
                                    start=(mc == 0), stop=(mc == MT - 1),
                                )
                            nc.vector.tensor_copy(ot[:, et, :], po)
                        nc.gpsimd.dma_start(
                            out=outT_r[:, j * 4 : (j + 1) * 4, osl], in_=ot
                        )
    nc.compile()
    return nc


_cached = {}


def _get_module(has_bias):
    if has_bias not in _cached:
        _cached[has_bias] = build_module(has_bias)
    return _cached[has_bias]


def _prep_inputs(x, Wv, Wg, bg, Wq, bq, Wo, has_bias):
    """Shard + lay out host-side. Returns per-core input dicts."""
    x = np.asarray(x, dtype=np.float32)
    Wvs = (np.asarray(Wv, dtype=np.float32) * SCALE).astype(BF16_NP)
    Wos = (np.asarray(Wo, dtype=np.float32) * SCALE).astype(BF16_NP)
    Wg = np.asarray(Wg, dtype=np.float32).astype(BF16_NP)
    Wq = np.asarray(Wq, dtype=np.float32).astype(BF16_NP)
    in_maps = []
    for c in range(N_CORES):
        b, h = c // 2, c % 2
        xTc = np.zeros((E, T), dtype=BF16_NP)
        start = h * OUT_T - HALO
        src = x[b, max(start, 0) : h * OUT_T + OUT_T].T.astype(BF16_NP)
        xTc[:, T - src.shape[1] :] = src
        m = {"xT": xTc, "Wvs": Wvs, "Wg": Wg, "Wq": Wq, "Wos": Wos}
        if has_bias:
            m["bg"] = np.ascontiguousarray(bg, dtype=np.float32)
            m["bq"] = np.ascontiguousarray(bq, dtype=np.float32)
        in_maps.append(m)
    return in_maps


def run(x, Wv, Wg, bg, Wq, bq, Wo, trace=False):
    bg = np.asarray(bg, dtype=np.float32)
    bq = np.asarray(bq, dtype=np.float32)
    has_bias = bool(np.any(bg)) or bool(np.any(bq))
    nc = _get_module(has_bias)
    in_maps = _prep_inputs(x, Wv, Wg, bg, Wq, bq, Wo, has_bias)
    res = run_bass_kernel_spmd(
        nc, in_maps, core_ids=list(range(N_CORES)), trace=trace
    )
    out = np.empty((B, S, E), dtype=np.float32)
    for c in range(N_CORES):
        b, h = c // 2, c % 2
        out[b, h * OUT_T : (h + 1) * OUT_T] = res.results[c]["outT"].T
    return out, res


def kernel(**inputs):
    out, _ = run(**inputs)
    return out


# revision 6
# speedup vs baseline: 1.0032x; 1.0032x over previous
"""Trainium2 Bass kernel for nn_Decay (gated decay-memory block).

  gate  = sigmoid(x @ Wg + bg)
  store = (x @ Wv) * gate * scale          scale = sqrt(1 - decay)
  mem   = decay-scan(store)                y_t = store_t + decay * y_{t-1}
  que   = sigmoid(x @ Wq + bq)
  out   = (mem * que * scale) @ Wo

Sharding (8 cores): core c handles batch b = c//2, token half h = c%2
(2048 output tokens each).  The decay scan needs history: each core
computes a 128-token halo before its token range (zero-padded for h=0,
so all cores run the identical program).  decay^128 ~ 1.4e-3, below the
bf16 noise floor.  No collectives.

All matmul operands are bf16 (measured: 512-row bf16 matmul = 216 ns vs
227 ns f32r; 256-row = 109 ns; accumulation stays f32 in PSUM; end-to-end
rel err ~3.5e-3 vs the 2e-2 gate).  x lives RESIDENT in SBUF as bf16
[E, T] (68 KB/partition), so each of the 4 m-quarter phases re-reads it
from SBUF at zero DMA cost — total HBM traffic drops to ~75 MB/core and
the DMA-burst stalls at phase boundaries disappear.

Layout: [feature (partitions), token (free)] everywhere.
 - projections:  out[m_tile, t_blk] = sum_ec Wx[ec, m_tile].T @ xres[ec, t_blk]
 - decay scan: DVE tensor_tensor_scan along the free (token) axis
 - matmul free dim TB=512 (one full PSUM bank) amortizes instruction
   overhead; the 128-token halo block runs as a cheap 128-row matmul.

Phases (x resident, weights double-buffered):
  A0..A3: m-quarter q of {Wv*scale, Wg, Wq} resident; computes
          pv, gate, store, mem(scan), que, l0 -> spill l0 (bf16)
  C:      Wo*scale fully resident in the freed weight buffers;
          outT[e, t] = sum_m Wo.T @ l0, one pass over l0.
"""

import sys

sys.path.insert(0, "/opt/trn_rl_repo")

import numpy as np
import ml_dtypes

import concourse.bass as bass
import concourse.tile as tile
from concourse import bacc, mybir
from concourse.bass_utils import run_bass_kernel_spmd

# Problem constants (hardcoded per harness contract)
B, S, E, M = 4, 4096, 2048, 2048
DECAY = 0.95
SCALE = float(np.sqrt(1.0 - DECAY))

N_CORES = 8
HALO = 128            # halo tokens ahead of each core's range
OUT_T = S // 2        # output tokens per core (2048)
T = OUT_T + HALO      # resident tokens per core (2176)
TB = 512              # token block (matmul free dim, one PSUM bank)
NTB = OUT_T // TB     # 4 output-token blocks
P = 128
EC = E // P           # 16 contraction chunks
MT = M // P           # 16 m tiles
MQ = 4                # m-quarters
MT_Q = MT // MQ       # 4 m-tiles per quarter
MQW = MT_Q * P        # 512
EQ = E // 4           # 512-wide output-e range per Wo tile
F32 = mybir.dt.float32
BF16 = mybir.dt.bfloat16
BF16_NP = ml_dtypes.bfloat16


def build_module(has_bias):
    nc = bacc.Bacc()

    xT_d = nc.dram_tensor("xT", [E, T], BF16, kind="ExternalInput")
    wv_d = nc.dram_tensor("Wvs", [E, M], BF16, kind="ExternalInput")
    wg_d = nc.dram_tensor("Wg", [E, M], BF16, kind="ExternalInput")
    wq_d = nc.dram_tensor("Wq", [E, M], BF16, kind="ExternalInput")
    wo_d = nc.dram_tensor("Wos", [M, E], BF16, kind="ExternalInput")
    if has_bias:
        bg_d = nc.dram_tensor("bg", [M], F32, kind="ExternalInput")
        bq_d = nc.dram_tensor("bq", [M], F32, kind="ExternalInput")
    outT_d = nc.dram_tensor("outT", [E, OUT_T], F32, kind="ExternalOutput")
    l0_buf = nc.dram_tensor("l0_buf", [M, OUT_T], BF16)  # internal spill

    with tile.TileContext(nc) as tc:
        with (
            tc.tile_pool(name="cp", bufs=1) as cp,
            tc.tile_pool(name="wvp", bufs=2) as wvp,
            tc.tile_pool(name="wgp", bufs=2) as wgp,
            tc.tile_pool(name="wqp", bufs=2) as wqp,
            tc.tile_pool(name="wsp", bufs=3) as wsp,
            tc.tile_pool(name="l0p", bufs=4) as l0p,
            tc.tile_pool(name="memp", bufs=2) as memp,
            tc.tile_pool(name="ps", bufs=2, space="PSUM") as ps,
        ):
            # consts: decay broadcast [:, :TB]; bg at [:, TB:TB+MT]; bq after
            consts = cp.tile([P, TB + 2 * MT], F32, tag="consts", name="consts")
            nc.vector.memset(consts[:, 0:TB], DECAY)
            if has_bias:
                nc.sync.dma_start(
                    out=consts[:, TB : TB + MT],
                    in_=bg_d.rearrange("(c p) -> p c", p=P),
                )
                nc.sync.dma_start(
                    out=consts[:, TB + MT : TB + 2 * MT],
                    in_=bq_d.rearrange("(c p) -> p c", p=P),
                )
            decay_t = consts[:, 0:TB]

            xT_r = xT_d.rearrange("(c p) t -> p c t", p=P)
            l0_r = l0_buf.rearrange("(c p) t -> p c t", p=P)
            outT_r = outT_d.rearrange("(c p) t -> p c t", p=P)

            w_tiles = {}
            # each weight stream gets its own DMA ring so startup loads
            # run in parallel (DMA here is descriptor-rate-bound); only
            # sync/scalar/gpsimd can issue DMAs
            RING = {"wv": nc.scalar, "wg": nc.gpsimd, "wq": nc.sync}

            def load_w(pool, dram, key, q, qname):
                msl = slice(q * MQW, (q + 1) * MQW)
                t = pool.tile([P, EC, MQW], BF16, tag=key, name=f"{qname}{q}")
                RING[key].dma_start(
                    out=t, in_=dram[:, msl].rearrange("(c p) m -> p c m", p=P)
                )
                w_tiles[(key, q)] = t

            def load_wo(pool, key, eq):
                # Wo e-quarter [P, MT, EQ] = 16KB, same size as an A tile
                esl = slice(eq * EQ, (eq + 1) * EQ)
                t = pool.tile([P, MT, EQ], BF16, tag=key, name=f"wo{eq}")
                RING[key].dma_start(
                    out=t, in_=wo_d[:, esl].rearrange("(c p) e -> p c e", p=P)
                )
                w_tiles[("o", eq)] = t

            with tc.tile_pool(name="xp", bufs=1) as xp:
                xres = xp.tile([P, EC, T], BF16, tag="xres", name="xres")
                # startup: halo + first block merged (one descriptor sweep);
                # weight quarters land in parallel on their own rings
                nc.sync.dma_start(
                    out=xres[:, :, 0 : HALO + TB],
                    in_=xT_r[:, :, 0 : HALO + TB],
                )
                load_w(wvp, wv_d, "wv", 0, "wv")
                load_w(wgp, wg_d, "wg", 0, "wg")
                load_w(wqp, wq_d, "wq", 0, "wq")
                nc.sync.dma_start(
                    out=xres[:, :, HALO + TB : T],
                    in_=xT_r[:, :, HALO + TB : T],
                )

                # ---- Phases A0..A3: quarter q of m ----
                for q in range(MQ):
                    wv = w_tiles[("wv", q)]
                    wg = w_tiles[("wg", q)]
                    wq = w_tiles[("wq", q)]

                    # halo block: pv/pg/scan only (cheap 128-row matmuls)
                    mem_h = memp.tile(
                        [P, MT_Q, TB], F32, tag="mem", name=f"memh{q}"
                    )
                    hsl = slice(0, HALO)
                    for mt in range(MT_Q):
                        wsl = slice(mt * P, (mt + 1) * P)
                        pv = ps.tile(
                            [P, TB], F32, tag="pv", bufs=5, name=f"pvh{q}_{mt}"
                        )
                        for ec in range(EC):
                            nc.tensor.matmul(
                                pv[:, hsl], lhsT=wv[:, ec, wsl],
                                rhs=xres[:, ec, hsl],
                                start=(ec == 0), stop=(ec == EC - 1),
                            )
                        pg = ps.tile(
                            [P, TB], F32, tag="pg", bufs=3, name=f"pgh{q}_{mt}"
                        )
                        for ec in range(EC):
                            nc.tensor.matmul(
                                pg[:, hsl], lhsT=wg[:, ec, wsl],
                                rhs=xres[:, ec, hsl],
                                start=(ec == 0), stop=(ec == EC - 1),
                            )
                        mtg = q * MT_Q + mt
                        ws = wsp.tile([P, 3, TB], F32, tag="ws", name=f"wsh{q}_{mt}")
                        gate, store = ws[:, 0, hsl], ws[:, 1, hsl]
                        nc.scalar.activation(
                            gate, pg[:, hsl], mybir.ActivationFunctionType.Sigmoid,
                            bias=consts[:, TB + mtg : TB + mtg + 1] if has_bias else 0.0,
                        )
                        nc.vector.tensor_mul(store, pv[:, hsl], gate)
                        nc.vector.tensor_tensor_scan(
                            mem_h[:, mt, hsl], decay_t[:, hsl], store,
                            initial=0.0,
                            op0=mybir.AluOpType.mult, op1=mybir.AluOpType.add,
                        )
                    mem_prev, prev_last = mem_h, HALO - 1

                    for tb in range(NTB):
                        xsl = slice(HALO + tb * TB, HALO + (tb + 1) * TB)
                        osl = slice(tb * TB, (tb + 1) * TB)
                        # spread next-phase weight loads across the phase
                        if q + 1 < MQ:
                            if tb == 1:
                                load_w(wvp, wv_d, "wv", q + 1, "wv")
                            elif tb == 2:
                                load_w(wgp, wg_d, "wg", q + 1, "wg")
                            elif tb == 3:
                                load_w(wqp, wq_d, "wq", q + 1, "wq")
                        else:
                            # Wo quarters into the A-weight buffers as they free
                            if tb == 1:
                                load_wo(wvp, "wv", 0)
                            elif tb == 2:
                                load_wo(wgp, "wg", 1)
                            elif tb == 3:
                                load_wo(wqp, "wq", 2)
                        mem_t = memp.tile(
                            [P, MT_Q, TB], F32, tag="mem", name=f"mem{q}_{tb}"
                        )
                        pvs = []
                        for mt in range(MT_Q):
                            wsl = slice(mt * P, (mt + 1) * P)
                            pv = ps.tile(
                                [P, TB], F32, tag="pv", bufs=5,
                                name=f"pv{q}_{tb}_{mt}",
                            )
                            for ec in range(EC):
                                nc.tensor.matmul(
                                    pv, lhsT=wv[:, ec, wsl], rhs=xres[:, ec, xsl],
                                    start=(ec == 0), stop=(ec == EC - 1),
                                )
                            pvs.append(pv)
                        wss = []
                        for mt in range(MT_Q):
                            mtg = q * MT_Q + mt
                            wsl = slice(mt * P, (mt + 1) * P)
                            pg = ps.tile(
                                [P, TB], F32, tag="pg", bufs=3,
                                name=f"pg{q}_{tb}_{mt}",
                            )
                            for ec in range(EC):
                                nc.tensor.matmul(
                                    pg, lhsT=wg[:, ec, wsl], rhs=xres[:, ec, xsl],
                                    start=(ec == 0), stop=(ec == EC - 1),
                                )
                            ws = wsp.tile(
                                [P, 3, TB], F32, tag="ws", name=f"ws{q}_{tb}_{mt}"
                            )
                            wss.append(ws)
                            gate, store = ws[:, 0, :], ws[:, 1, :]
                            nc.scalar.activation(
                                gate, pg, mybir.ActivationFunctionType.Sigmoid,
                                bias=consts[:, TB + mtg : TB + mtg + 1]
                                if has_bias else 0.0,
                            )
                            nc.vector.tensor_mul(store, pvs[mt], gate)
                            nc.vector.tensor_tensor_scan(
                                mem_t[:, mt, :], decay_t, store,
                                initial=mem_prev[:, mt, prev_last : prev_last + 1],
                                op0=mybir.AluOpType.mult, op1=mybir.AluOpType.add,
                            )
                        for mt in range(MT_Q):
                            mtg = q * MT_Q + mt
                            wsl = slice(mt * P, (mt + 1) * P)
                            pq = ps.tile(
                                [P, TB], F32, tag="pv", bufs=5,
                                name=f"pq{q}_{tb}_{mt}",
                            )
                            for ec in range(EC):
                                nc.tensor.matmul(
                                    pq, lhsT=wq[:, ec, wsl], rhs=xres[:, ec, xsl],
                                    start=(ec == 0), stop=(ec == EC - 1),
                                )
                            que = wss[mt][:, 2, :]
                            nc.scalar.activation(
                                que, pq, mybir.ActivationFunctionType.Sigmoid,
                                bias=consts[:, TB + MT + mtg : TB + MT + mtg + 1]
                                if has_bias else 0.0,
                            )
                            l0 = l0p.tile([P, TB], BF16, tag="l0",
                                          name=f"l0{q}_{tb}_{mt}")
                            nc.vector.tensor_mul(l0, mem_t[:, mt, :], que)
                            nc.gpsimd.dma_start(
                                out=l0_r[:, mtg : mtg + 1, osl],
                                in_=l0.unsqueeze(1),
                            )
                        mem_prev, prev_last = mem_t, TB - 1

            # ---- Phase C: output projection, Wo fully resident ----
            # wo3 takes the second wv-tag buffer (freed at A3's end); its DMA
            # overlaps the first three e-quarters' matmuls at tb=0.
            with tc.tile_pool(name="ltp", bufs=3) as ltp:
                for tb in range(NTB):
                    osl = slice(tb * TB, (tb + 1) * TB)
                    lt = ltp.tile([P, MT, TB], BF16, tag="lt", name=f"lt{tb}")
                    if tb == 0:
                        # chunked so accumulation starts as chunks land
                        for mc4 in range(0, MT, 4):
                            nc.sync.dma_start(
                                out=lt[:, mc4 : mc4 + 4, :],
                                in_=l0_r[:, mc4 : mc4 + 4, osl],
                            )
                        load_wo(wvp, "wv", 3)
                    else:
                        nc.sync.dma_start(out=lt, in_=l0_r[:, :, osl])
                    for j in range(4):
                        wo = w_tiles[("o", j)]
                        ot = memp.tile([P, 4, TB], F32, tag="mem", name=f"ot{tb}_{j}")
                        ring = (nc.gpsimd, nc.scalar)[j % 2]
                        for et in range(4):
                            po = ps.tile(
                                [P, TB], F32, tag="pg", bufs=3,
                                name=f"po{tb}_{j}_{et}",
                            )
                            for mc in range(MT):
                                nc.tensor.matmul(
                                    po, lhsT=wo[:, mc, et * P : (et + 1) * P],
                                    rhs=lt[:, mc, :],
                                    start=(mc == 0), stop=(mc == MT - 1),
                                )
                            nc.vector.tensor_copy(ot[:, et, :], po)
                            ring.dma_start(
                                out=outT_r[:, j * 4 + et : j * 4 + et + 1, osl],
                                in_=ot[:, et : et + 1, :],
                            )
    nc.compile()
    return nc


_cached = {}


def _get_module(has_bias):
    if has_bias not in _cached:
        _cached[has_bias] = build_module(has_bias)
    return _cached[has_bias]


def _prep_inputs(x, Wv, Wg, bg, Wq, bq, Wo, has_bias):
    """Shard + lay out host-side. Returns per-core input dicts."""
    x = np.asarray(x, dtype=np.float32)
    Wvs = (np.asarray(Wv, dtype=np.float32) * SCALE).astype(BF16_NP)
    Wos = (np.asarray(Wo, dtype=np.float32) * SCALE).astype(BF16_NP)
    Wg = np.asarray(Wg, dtype=np.float32).astype(BF16_NP)
    Wq = np.asarray(Wq, dtype=np.float32).astype(BF16_NP)
    in_maps = []
    for c in range(N_CORES):
        b, h = c // 2, c % 2
        xTc = np.zeros((E, T), dtype=BF16_NP)
        start = h * OUT_T - HALO
        src = x[b, max(start, 0) : h * OUT_T + OUT_T].T.astype(BF16_NP)
        xTc[:, T - src.shape[1] :] = src
        m = {"xT": xTc, "Wvs": Wvs, "Wg": Wg, "Wq": Wq, "Wos": Wos}
        if has_bias:
            m["bg"] = np.ascontiguousarray(bg, dtype=np.float32)
            m["bq"] = np.ascontiguousarray(bq, dtype=np.float32)
        in_maps.append(m)
    return in_maps


def run(x, Wv, Wg, bg, Wq, bq, Wo, trace=False):
    bg = np.asarray(bg, dtype=np.float32)
    bq = np.asarray(bq, dtype=np.float32)
    has_bias = bool(np.any(bg)) or bool(np.any(bq))
    nc = _get_module(has_bias)
    in_maps = _prep_inputs(x, Wv, Wg, bg, Wq, bq, Wo, has_bias)
    res = run_bass_kernel_spmd(
        nc, in_maps, core_ids=list(range(N_CORES)), trace=trace
    )
    out = np.empty((B, S, E), dtype=np.float32)
    for c in range(N_CORES):
        b, h = c // 2, c % 2
        out[b, h * OUT_T : (h + 1) * OUT_T] = res.results[c]["outT"].T
    return out, res


def kernel(**inputs):
    out, _ = run(**inputs)
    return out
